# revision 17
# baseline (speedup 1.0000x reference)
"""Trainium2 Bass kernel for LocalGraphProjection (perceptual feature pooling).

Pipeline per point: project through 3 cameras, bilinear-sample 3 feature
pyramid levels per view (16/32/64 ch), concat -> [N,112] per view, then
max/mean/std across views -> [N, 3+336] output.

v2 strategy (vs v1: 9 gathers/point, 1 queue, reduce-X combine):
  - ONE mega-table per view, keyed (i0, jp=j0//2): the L1/L2 cells are
    fully determined by the L0 cell (floor(x/2) == floor(x)//2 for clipped
    coords), so a single 1024B entry carries all 3 levels' corner patches:
    [L0: 16ch x 8 slots][L1: 32ch x 4][L2: 64ch x 4] = 512 f16.
    One dma_gather index per (point, view): 3x fewer descriptors.
  - Gathers round-robin the 4 SWDGE queues (rings process in parallel,
    measured ~3.8x); queue = counter%4 aligns with Tile's 8 DMASW sem
    lanes so each sem lane sees a single queue.
  - Combine on Vector via broadcast-middle multiply + pairwise halving
    adds (measured ~2x faster than tensor_reduce-X).
  - fmax / view-sum on GpSimd; squares/sqrt/scales on Scalar; variance in
    uncentered form sqrt(E[x^2] - mean^2) to shorten the Vector chain.
"""

import numpy as np

import concourse.bass as bass
import concourse.bacc as bacc
import concourse.mybir as mybir
from concourse.tile import TileContext
from concourse.bass_utils import run_bass_kernel_spmd
from concourse import library_config

F32 = mybir.dt.float32
F16 = mybir.dt.float16
I16 = mybir.dt.int16
I32 = mybir.dt.int32
ALU = mybir.AluOpType
ACTF = mybir.ActivationFunctionType

PI = float(np.pi)

N_PTS = 262144
N_CORES = 8
N_CORE_PTS = N_PTS // N_CORES  # 32768
P = 128
M = N_CORE_PTS // P  # 256 slots per partition

# points per gather tile
T = 1024
MT = T // P  # 8
NT = M // MT  # 32

TAB_ROWS = 224 * 112  # 25088
ENT = 512  # f16 per mega-entry (1024B)
N_QUEUES = 4


# ----------------------------------------------------------------- host math
def _camera_affines(cameras: np.ndarray):
    """Per-view affine pc = coord @ A_v + b_v, in float64 (mirrors reference)."""
    cams = cameras.astype(np.float64)

    def cm(param):
        theta = param[0] * (PI / 180.0)
        camy = param[3] * np.sin(param[1] * PI / 180.0)
        lens = param[3] * np.cos(param[1] * PI / 180.0)
        camx = lens * np.cos(theta)
        camz = lens * np.sin(theta)
        Zv = np.array([camx, camy, camz])
        Yv = np.array([camy * np.cos(theta + PI), lens, camy * np.sin(theta + PI)])
        Xv = np.cross(Yv, Zv)
        c = np.stack(
            [Xv / np.linalg.norm(Xv), Yv / np.linalg.norm(Yv), Zv / np.linalg.norm(Zv)]
        )
        return c, Zv

    c0, o0 = cm(cams[0])
    M0 = np.linalg.inv(c0.T)
    A, B = [], []
    for v in range(3):
        cv, ov = cm(cams[v])
        A.append(M0 @ cv.T)            # [3,3]
        B.append((o0 - ov) @ cv.T)     # [3]
    return A, B


def _build_affine_plane(cameras: np.ndarray) -> np.ndarray:
    """[P, 40] fp32: per view v, 12 consts at col v*13:
    [a00,a10,a20,b0, -a01,-a11,-a21,-b1, -a02,-a12,-a22,-b2]."""
    A, B = _camera_affines(cameras)
    row = np.zeros(40, np.float32)
    for v in range(3):
        a, b = A[v], B[v]
        base = v * 13
        row[base + 0 : base + 3] = a[:, 0]
        row[base + 3] = b[0]
        row[base + 4 : base + 7] = -a[:, 1]
        row[base + 7] = -b[1]
        row[base + 8 : base + 11] = -a[:, 2]
        row[base + 11] = -b[2]
    return np.tile(row[None, :], (P, 1))


def _build_megatables(img_feat0, img_feat1, img_feat2):
    """Per view: [25088, 512] f16 entries keyed (i0, jp).

    Entry layout (channel-major per level, slot s innermost):
      [  0:128] L0: c=0..15, s=r*4+cw; rows {i0, i0+1}c, wincols {2jp+cw}c
      [128:256] L1: c=0..31, s=r*2+cc; cell (i0//2, jp)
      [256:512] L2: c=0..63, s=r*2+cc; cell (i0//4, jp//2)
    """
    f0 = np.asarray(img_feat0)
    f1 = np.asarray(img_feat1)
    f2 = np.asarray(img_feat2)
    tabs = {}
    for v in range(3):
        F0, F1, F2 = f0[v], f1[v], f2[v]  # [224,224,16] [112,112,32] [56,56,64]
        # L0: [224 rows, 112 jp, 16c, 8s]
        rows = np.stack([np.arange(224), np.minimum(np.arange(224) + 1, 223)], 1)
        cols = np.minimum(
            2 * np.arange(112)[:, None] + np.arange(4)[None, :], 223
        )  # [112, 4]
        t = F0[rows]            # [224, 2, 224, 16]
        t = t[:, :, cols]       # [224, 2, 112, 4, 16]
        l0 = np.transpose(t, (0, 2, 4, 1, 3)).reshape(224, 112, 128)
        # L1 base: [112, 112, 32c, 4s]
        rows1 = np.stack([np.arange(112), np.minimum(np.arange(112) + 1, 111)], 1)
        cols1 = np.stack([np.arange(112), np.minimum(np.arange(112) + 1, 111)], 1)
        t = F1[rows1]           # [112, 2, 112, 32]
        t = t[:, :, cols1]      # [112, 2, 112, 2, 32]
        l1 = np.transpose(t, (0, 2, 4, 1, 3)).reshape(112, 112, 128)
        l1 = np.repeat(l1, 2, axis=0)  # [224, 112, 128]
        # L2 base: [56, 56, 64c, 4s]
        rows2 = np.stack([np.arange(56), np.minimum(np.arange(56) + 1, 55)], 1)
        cols2 = np.stack([np.arange(56), np.minimum(np.arange(56) + 1, 55)], 1)
        t = F2[rows2]           # [56, 2, 56, 64]
        t = t[:, :, cols2]      # [56, 2, 56, 2, 64]
        l2 = np.transpose(t, (0, 2, 4, 1, 3)).reshape(56, 56, 256)
        l2 = np.repeat(np.repeat(l2, 4, axis=0), 2, axis=1)  # [224, 112, 256]
        tab = np.concatenate([l0, l1, l2], axis=2)  # [224, 112, 512]
        tabs[v] = np.ascontiguousarray(tab.reshape(TAB_ROWS, ENT).astype(np.float16))
    return tabs


# ------------------------------------------------------------- device kernel
def emit_body(nc, tc, pools, dram, m_total=M, mt=MT):
    """Emit the whole per-core program inside an open TileContext."""
    nt = m_total // mt
    mt2 = 2 * mt
    nt2 = m_total // mt2
    sc, wpool, gpool, fpool, opool, ipool = (
        pools["sc"], pools["w"], pools["g"], pools["f"], pools["o"], pools["i"],
    )
    coords_d, afp_d, tabs_d, out_d = (
        dram["coords"], dram["afp"], dram["tabs"], dram["out"],
    )

    V = nc.vector
    G = nc.gpsimd
    S = nc.scalar
    IO = nc.sync

    G.load_library(library_config.mlp)
    nidx_reg = G.alloc_register("nidx")
    G.reg_mov(nidx_reg, P * mt)

    # ---- preload
    coords_sb = sc.tile([P, 3, m_total], F32, tag="coords", name="coords_sb")
    IO.dma_start(out=coords_sb[:, :, :], in_=coords_d[:, :, :].transpose([1, 0, 2]))
    afp = sc.tile([P, 40], F32, tag="afp", name="afp_sb")
    IO.dma_start(out=afp[:, :], in_=afp_d[:, :])

    def ap_s(col):  # [P,1] scalar AP
        return afp[:, col : col + 1]

    cx = coords_sb[:, 0, :]
    cy = coords_sb[:, 1, :]
    cz = coords_sb[:, 2, :]

    # ---- whole-core per-point math (fp32, [P, m_total])
    w_tiles = {}   # v -> [P, m_total, 16] fp16 slot weights
    idx_f = sc.tile([P, 3, m_total], F32, tag="idxf", name="idxf")

    def newt(tag, dt=F32, d3=None, pool=sc):
        shape = [P, m_total] if d3 is None else [P, m_total, d3]
        return pool.tile(shape, dt, tag=tag, name=tag)

    for v in range(3):
        base = v * 13
        X = newt("Xs")
        nY = newt("nYs")
        nZ = newt("nZs")
        for out_t, off in ((X, 0), (nY, 4), (nZ, 8)):
            V.tensor_scalar(out_t[:, :], cx, ap_s(base + off + 0), None, ALU.mult)
            V.scalar_tensor_tensor(
                out_t[:, :], cy, ap_s(base + off + 1), out_t[:, :], ALU.mult, ALU.add
            )
            V.scalar_tensor_tensor(
                out_t[:, :], cz, ap_s(base + off + 2), out_t[:, :], ALU.mult, ALU.add
            )
            V.tensor_scalar(out_t[:, :], out_t[:, :], ap_s(base + off + 3), None, ALU.add)
        rz = newt("rzs")
        V.reciprocal(rz[:, :], nZ[:, :])
        h = newt("hs")
        w_ = newt("ws")
        V.tensor_tensor(h[:, :], nY[:, :], rz[:, :], ALU.mult)
        V.tensor_scalar(h[:, :], h[:, :], 248.0, 112.0, ALU.mult, ALU.add)
        V.tensor_scalar(h[:, :], h[:, :], 0.0, 223.0, ALU.max, ALU.min)
        V.tensor_tensor(w_[:, :], X[:, :], rz[:, :], ALU.mult)
        V.tensor_scalar(w_[:, :], w_[:, :], 248.0, 112.0, ALU.mult, ALU.add)
        V.tensor_scalar(w_[:, :], w_[:, :], 0.0, 223.0, ALU.max, ALU.min)

        # floors: i0 = floor(h), fx = h - i0 ; j0 = floor(w), fy = w - j0
        fx = newt("fxs")
        fy = newt("fys")
        i0 = newt("i0s")
        j0 = newt("j0s")
        xi = newt("xis", dt=I32)
        for (xx, x1x, fxx) in ((h, i0, fx), (w_, j0, fy)):
            V.tensor_copy(xi[:, :], xx[:, :])
            V.tensor_copy(x1x[:, :], xi[:, :])
            V.tensor_tensor(fxx[:, :], x1x[:, :], xx[:, :], ALU.is_gt)
            V.tensor_tensor(x1x[:, :], x1x[:, :], fxx[:, :], ALU.subtract)
            V.tensor_tensor(fxx[:, :], xx[:, :], x1x[:, :], ALU.subtract)

        # parities of i0 (mod 2, mod 4) and j0 (mod 2, mod 4)
        pi2 = newt("pi2s")
        pi4 = newt("pi4s")
        pj2 = newt("pj2s")
        pj4 = newt("pj4s")
        for (src, dst, msk) in ((i0, pi2, 1), (i0, pi4, 3), (j0, pj2, 1), (j0, pj4, 3)):
            V.tensor_copy(xi[:, :], src[:, :])
            V.tensor_scalar(xi[:, :], xi[:, :], msk, None, ALU.bitwise_and)
            V.tensor_copy(dst[:, :], xi[:, :])

        # jp = (j0 - pj2)/2 ; idx = i0*112 + jp
        jp = newt("jps")
        V.tensor_tensor(jp[:, :], j0[:, :], pj2[:, :], ALU.subtract)
        V.tensor_scalar(jp[:, :], jp[:, :], 0.5, None, ALU.mult)
        V.scalar_tensor_tensor(
            idx_f[:, v, :], i0[:, :], 112.0, jp[:, :], ALU.mult, ALU.add
        )

        # row weights L0: wx1 = (fx>0) - fx ; wx2 = fx
        wx1 = newt("wx1s")
        wy1 = newt("wy1s")
        V.tensor_scalar(wx1[:, :], fx[:, :], 0.0, None, ALU.is_gt)
        V.tensor_tensor(wx1[:, :], wx1[:, :], fx[:, :], ALU.subtract)
        V.tensor_scalar(wy1[:, :], fy[:, :], 0.0, None, ALU.is_gt)
        V.tensor_tensor(wy1[:, :], wy1[:, :], fy[:, :], ALU.subtract)

        # L0 window-column weights: a0 = wy1*(1-pj2); a1 = wy1*pj2 + fy*(1-pj2);
        # a2 = fy*pj2 ; col 3 = 0
        t1 = newt("i0s")
        t2 = newt("j0s")
        a0 = newt("jps")
        a1 = newt("a1s")
        V.tensor_tensor(t1[:, :], wy1[:, :], pj2[:, :], ALU.mult)
        V.tensor_tensor(a0[:, :], wy1[:, :], t1[:, :], ALU.subtract)
        V.tensor_tensor(t2[:, :], fy[:, :], pj2[:, :], ALU.mult)
        V.tensor_tensor(a1[:, :], t1[:, :], fy[:, :], ALU.add)
        V.tensor_tensor(a1[:, :], a1[:, :], t2[:, :], ALU.subtract)

        # L1 fracs: fx1 = (fx+pi2)/2, fy1 = (fy+pj2)/2; L2: /4 with mod-4 parities
        fx1 = newt("Xs")
        fy1 = newt("nYs")
        fx2 = newt("nZs")
        fy2 = newt("rzs")
        V.tensor_tensor(fx1[:, :], fx[:, :], pi2[:, :], ALU.add)
        V.tensor_scalar(fx1[:, :], fx1[:, :], 0.5, None, ALU.mult)
        V.tensor_tensor(fy1[:, :], fy[:, :], pj2[:, :], ALU.add)
        V.tensor_scalar(fy1[:, :], fy1[:, :], 0.5, None, ALU.mult)
        V.tensor_tensor(fx2[:, :], fx[:, :], pi4[:, :], ALU.add)
        V.tensor_scalar(fx2[:, :], fx2[:, :], 0.25, None, ALU.mult)
        V.tensor_tensor(fy2[:, :], fy[:, :], pj4[:, :], ALU.add)
        V.tensor_scalar(fy2[:, :], fy2[:, :], 0.25, None, ALU.mult)
        wx1_1 = newt("hs")
        wy1_1 = newt("ws")
        wx1_2 = newt("wx12s")
        wy1_2 = newt("wy12s")
        for (f, o) in ((fx1, wx1_1), (fy1, wy1_1), (fx2, wx1_2), (fy2, wy1_2)):
            V.tensor_scalar(o[:, :], f[:, :], 0.0, None, ALU.is_gt)
            V.tensor_tensor(o[:, :], o[:, :], f[:, :], ALU.subtract)

        # weight slots, tiled layout [P, nt, 3, mt, 16] so the combine can
        # merge (view, m) into one uniform-stride dim per tile
        if "w_all" not in w_tiles:
            w_tiles["w_all"] = wpool.tile(
                [P, nt2, 3, mt2, 16], F16, tag="w_all", name="w_all"
            )
        w_all = w_tiles["w_all"]

        def wslot(s):
            return w_all[:, :, v, :, s]

        def asT(x):  # [P, m_total] -> [P, nt2, mt2]
            return x[:, :].rearrange("p (t m) -> p t m", m=mt2)

        # L0 slots 0..7: s = r*4+cw; rows (wx1, fx) x cols (a0, a1, t2, 0)
        for r, rw in ((0, wx1), (1, fx)):
            for c, cw in ((0, a0), (1, a1), (2, t2)):
                V.tensor_tensor(wslot(r * 4 + c), asT(rw), asT(cw), ALU.mult)
        V.tensor_scalar(wslot(3), asT(pj2), 0.0, None, ALU.mult)
        V.tensor_scalar(wslot(7), asT(pj2), 0.0, None, ALU.mult)
        # L1 slots 8..11: (wx1_1*wy1_1, wx1_1*fy1, fx1*wy1_1, fx1*fy1)
        V.tensor_tensor(wslot(8), asT(wx1_1), asT(wy1_1), ALU.mult)
        V.tensor_tensor(wslot(9), asT(wx1_1), asT(fy1), ALU.mult)
        V.tensor_tensor(wslot(10), asT(fx1), asT(wy1_1), ALU.mult)
        V.tensor_tensor(wslot(11), asT(fx1), asT(fy1), ALU.mult)
        # L2 slots 12..15
        V.tensor_tensor(wslot(12), asT(wx1_2), asT(wy1_2), ALU.mult)
        V.tensor_tensor(wslot(13), asT(wx1_2), asT(fy2), ALU.mult)
        V.tensor_tensor(wslot(14), asT(fx2), asT(wy1_2), ALU.mult)
        V.tensor_tensor(wslot(15), asT(fx2), asT(fy2), ALU.mult)

    # cast indices fp32 -> int16
    idx_i = ipool.tile([P, 3, m_total], I16, tag="idxi", name="idxi")
    V.tensor_copy(idx_i[:, :, :].rearrange("p j m -> p (j m)"),
                  idx_f[:, :, :].rearrange("p j m -> p (j m)"))

    # ---- whole-core idx wrap: g=(m*128+p) at [g%16, g//16] per tile chunk,
    # replicated across the 128 partitions
    i32a = gpool.tile([32, 3, m_total, 4], I16, tag="g", name="i32a", bufs=2)
    for r4 in range(4):
        V.tensor_copy(i32a[:, :, :, r4], idx_i[32 * r4 : 32 * (r4 + 1), :, :])
    i16b = gpool.tile([16, 3, m_total, 4], I16, tag="g", name="i16b", bufs=2)
    IO.dma_start(out=i16b[:, :, :, :], in_=i32a[16:32, :, :, :])
    wr_all = ipool.tile([P, 3, 8 * m_total], I16, tag="wr", name="wr")
    wr6 = wr_all.rearrange("p j (t m k r2) -> p j t m k r2", m=mt, k=4, r2=2)
    V.tensor_copy(
        wr6[0:16, :, :, :, :, 0],
        i32a[0:16, :, :, :].rearrange("q j (t m) k -> q j t m k", m=mt),
    )
    V.tensor_copy(
        wr6[0:16, :, :, :, :, 1],
        i16b[:, :, :, :].rearrange("q j (t m) k -> q j t m k", m=mt),
    )
    IO.dma_start(out=wr_all[16:32, :, :], in_=wr_all[0:16, :, :])
    IO.dma_start(out=wr_all[32:64, :, :], in_=wr_all[0:32, :, :])
    IO.dma_start(out=wr_all[64:128, :, :], in_=wr_all[0:64, :, :])

    w_all = w_tiles["w_all"]
    VM = 3 * mt
    SQ3 = 1.0 / float(np.sqrt(3.0))

    # ---- per super-tile (2048 pts): 3x 2048-idx gathers, wide combine,
    # half-tile stats, store
    VM = 3 * mt2
    SQ3 = 1.0 / float(np.sqrt(3.0))
    gq = [0]  # queue = counter%4 keeps Tile DMASW sem lanes queue-pure
    for t in range(nt2):
        gt = gpool.tile([P, 3, mt2, ENT], F16, tag="g", name="g", bufs=2)
        for v in range(3):
            for hh in range(2):
                G.dma_gather(
                    gt[:, v, hh * mt : (hh + 1) * mt, :],
                    tabs_d[v][:, :],
                    wr_all[:, v, (2 * t + hh) * 8 * mt : (2 * t + hh + 1) * 8 * mt],
                    P * mt,
                    nidx_reg,
                    ENT,
                    queue_num=gq[0] % N_QUEUES,
                )
                gq[0] += 1
        gvm = gt[:, :, :, :].rearrange("p v m e -> p (v m) e")
        wt_t = w_all[:, t, :, :, :]
        g0 = gvm[:, :, 0:128].rearrange("p n (c s) -> p n c s", s=8)
        g12 = gvm[:, :, 128:512].rearrange("p n (c s) -> p n c s", s=4)
        g1 = gvm[:, :, 128:256].rearrange("p n (c s) -> p n c s", s=4)
        g2 = gvm[:, :, 256:512].rearrange("p n (c s) -> p n c s", s=4)
        # per-view multiplies so each view combines as soon as its gathers land
        for v in range(3):
            gv = gt[:, v, :, :]
            wv = wt_t[:, v, :, :]
            V.tensor_tensor(
                gv[:, :, 0:128].rearrange("p m (c s) -> p m c s", s=8),
                gv[:, :, 0:128].rearrange("p m (c s) -> p m c s", s=8),
                wv[:, :, 0:8].unsqueeze(2).broadcast_to([P, mt2, 16, 8]),
                ALU.mult)
            V.tensor_tensor(
                gv[:, :, 128:256].rearrange("p m (c s) -> p m c s", s=4),
                gv[:, :, 128:256].rearrange("p m (c s) -> p m c s", s=4),
                wv[:, :, 8:12].unsqueeze(2).broadcast_to([P, mt2, 32, 4]),
                ALU.mult)
            V.tensor_tensor(
                gv[:, :, 256:512].rearrange("p m (c s) -> p m c s", s=4),
                gv[:, :, 256:512].rearrange("p m (c s) -> p m c s", s=4),
                wv[:, :, 12:16].unsqueeze(2).broadcast_to([P, mt2, 64, 4]),
                ALU.mult)
        F_t = fpool.tile([P, 3, mt2, 112], F16, tag="F", name="F")
        Fvm = F_t[:, :, :, :].rearrange("p v m c -> p (v m) c")
        V.tensor_tensor(g0[:, :, :, 0:4], g0[:, :, :, 0:4], g0[:, :, :, 4:8],
                        ALU.add)
        V.tensor_tensor(g0[:, :, :, 0:2], g0[:, :, :, 0:2], g0[:, :, :, 2:4],
                        ALU.add)
        V.tensor_tensor(Fvm[:, :, 0:16], g0[:, :, :, 0], g0[:, :, :, 1], ALU.add)
        V.tensor_tensor(g12[:, :, :, 0:2], g12[:, :, :, 0:2], g12[:, :, :, 2:4],
                        ALU.add)
        V.tensor_tensor(Fvm[:, :, 16:112], g12[:, :, :, 0], g12[:, :, :, 1],
                        ALU.add)

        # ---- stats across views (full super-tile width)
        sl = slice(t * mt2, (t + 1) * mt2)
        out_t = opool.tile([P, mt2, 336], F16, tag="out", name="out_t", bufs=1)
        F0 = F_t[:, 0, :, :]
        F1 = F_t[:, 1, :, :]
        F2 = F_t[:, 2, :, :]
        fmax = out_t[:, :, 0:112]
        V.tensor_tensor(fmax, F0, F1, ALU.max)
        V.tensor_tensor(fmax, fmax, F2, ALU.max)
        ssum = fpool.tile([P, mt2, 112], F16, tag="ssum", name="ssum", bufs=1)
        V.tensor_tensor(ssum[:, :, :], F0, F1, ALU.add)
        V.tensor_tensor(ssum[:, :, :], ssum[:, :, :], F2, ALU.add)
        fmean = out_t[:, :, 112:224]
        S.activation(fmean, ssum[:, :, :], ACTF.Copy, scale=1.0 / 3.0)
        sqa = fpool.tile([P, mt2, 112], F16, tag="sqa", name="sqa")
        ssq = fpool.tile([P, mt2, 112], F16, tag="ssq", name="ssq", bufs=1)
        S.activation(ssq[:, :, :], F0, ACTF.Square, scale=SQ3)
        S.activation(sqa[:, :, :], F1, ACTF.Square, scale=SQ3)
        V.tensor_tensor(ssq[:, :, :], ssq[:, :, :], sqa[:, :, :], ALU.add)
        S.activation(sqa[:, :, :], F2, ACTF.Square, scale=SQ3)
        V.tensor_tensor(ssq[:, :, :], ssq[:, :, :], sqa[:, :, :], ALU.add)
        m2 = fpool.tile([P, mt2, 112], F16, tag="sqa", name="m2")
        S.activation(m2[:, :, :], fmean, ACTF.Square)
        V.tensor_tensor(ssq[:, :, :], ssq[:, :, :], m2[:, :, :], ALU.subtract)
        V.tensor_scalar(ssq[:, :, :], ssq[:, :, :], 0.0, None, ALU.max)
        S.activation(out_t[:, :, 224:336], ssq[:, :, :], ACTF.Sqrt)

        IO.dma_start(out=out_d[:, sl, :], in_=out_t[:, :, :])


def build_kernel(m_total=M, mt=MT, tab_rows=TAB_ROWS):
    """Build the Bass module. Returns nc."""
    nc = bacc.Bacc("TRN2", num_swdge_queues=N_QUEUES, dynamic_dma_scratch_size=16384)
    coords = nc.dram_tensor("coords", [3, P, m_total], F32, kind="ExternalInput")
    afp = nc.dram_tensor("afp", [P, 40], F32, kind="ExternalInput")
    tabs = {}
    for v in range(3):
        tabs[v] = nc.dram_tensor(
            f"tab{v}", [tab_rows, ENT], F16, kind="ExternalInput"
        )
    out = nc.dram_tensor("out", [P, m_total, 336], F16, kind="ExternalOutput")

    with nc.allow_low_precision("fp16 sampling kernel"), TileContext(nc) as tc:
        import contextlib

        stack = contextlib.ExitStack()
        pools = {
            "sc": stack.enter_context(tc.tile_pool(name="sc", bufs=1)),
            "w": stack.enter_context(tc.tile_pool(name="w", bufs=1)),
            "g": stack.enter_context(tc.tile_pool(name="g", bufs=2)),
            "f": stack.enter_context(tc.tile_pool(name="f", bufs=2)),
            "o": stack.enter_context(tc.tile_pool(name="o", bufs=2)),
            "i": stack.enter_context(tc.tile_pool(name="i", bufs=1)),
        }
        dram = {
            "coords": coords.ap(),
            "afp": afp.ap(),
            "tabs": {v: t.ap() for v, t in tabs.items()},
            "out": out.ap(),
        }
        with stack:
            emit_body(nc, tc, pools, dram, m_total=m_total, mt=mt)
    nc.compile()
    return nc


# ------------------------------------------------------------------ frontend
_NC_CACHE = {}
TRACE = False
LAST_RES = [None]


def _get_nc():
    if "nc" not in _NC_CACHE:
        _NC_CACHE["nc"] = build_kernel()
    return _NC_CACHE["nc"]


def kernel(coord, img_feat0, img_feat1, img_feat2, cameras):
    coord = np.asarray(coord, np.float32)
    afp = _build_affine_plane(np.asarray(cameras, np.float32))
    tabs = _build_megatables(img_feat0, img_feat1, img_feat2)

    nc = _get_nc()
    in_maps = []
    for k in range(N_CORES):
        shard = coord[k * N_CORE_PTS : (k + 1) * N_CORE_PTS]  # [32768, 3]
        cs = np.ascontiguousarray(
            shard.reshape(P, M, 3).transpose(2, 0, 1)
        )  # [3, P, M]
        im = {"coords": cs, "afp": afp}
        for v in range(3):
            im[f"tab{v}"] = tabs[v]
        in_maps.append(im)

    res = run_bass_kernel_spmd(
        nc, in_maps, core_ids=list(range(N_CORES)), trace=TRACE
    )
    LAST_RES[0] = res
    stats = np.concatenate(
        [res.results[k]["out"].reshape(N_CORE_PTS, 336) for k in range(N_CORES)], 0
    ).astype(np.float32)
    return np.concatenate([coord, stats], axis=1)


# revision 18
# speedup vs baseline: 1.0709x; 1.0709x over previous
"""Trainium2 Bass kernel for LocalGraphProjection (perceptual feature pooling).

Pipeline per point: project through 3 cameras, bilinear-sample 3 feature
pyramid levels per view (16/32/64 ch), concat -> [N,112] per view, then
max/mean/std across views -> [N, 3+336] output.

v2 strategy (vs v1: 9 gathers/point, 1 queue, reduce-X combine):
  - ONE mega-table per view, keyed (i0, jp=j0//2): the L1/L2 cells are
    fully determined by the L0 cell (floor(x/2) == floor(x)//2 for clipped
    coords), so a single 1024B entry carries all 3 levels' corner patches:
    [L0: 16ch x 8 slots][L1: 32ch x 4][L2: 64ch x 4] = 512 f16.
    One dma_gather index per (point, view): 3x fewer descriptors.
  - Gathers round-robin the 4 SWDGE queues (rings process in parallel,
    measured ~3.8x); queue = counter%4 aligns with Tile's 8 DMASW sem
    lanes so each sem lane sees a single queue.
  - Combine on Vector via broadcast-middle multiply + pairwise halving
    adds (measured ~2x faster than tensor_reduce-X).
  - fmax / view-sum on GpSimd; squares/sqrt/scales on Scalar; variance in
    uncentered form sqrt(E[x^2] - mean^2) to shorten the Vector chain.
"""

import numpy as np

import concourse.bass as bass
import concourse.bacc as bacc
import concourse.mybir as mybir
from concourse.tile import TileContext
from concourse.bass_utils import run_bass_kernel_spmd
from concourse import library_config

F32 = mybir.dt.float32
F16 = mybir.dt.float16
I16 = mybir.dt.int16
I32 = mybir.dt.int32
ALU = mybir.AluOpType
ACTF = mybir.ActivationFunctionType

PI = float(np.pi)

N_PTS = 262144
N_CORES = 8
N_CORE_PTS = N_PTS // N_CORES  # 32768
P = 128
M = N_CORE_PTS // P  # 256 slots per partition

# points per gather tile
T = 1024
MT = T // P  # 8
NT = M // MT  # 32

TAB_ROWS = 224 * 112  # 25088
ENT = 512  # f16 per mega-entry (1024B)
N_QUEUES = 4


# ----------------------------------------------------------------- host math
def _camera_affines(cameras: np.ndarray):
    """Per-view affine pc = coord @ A_v + b_v, in float64 (mirrors reference)."""
    cams = cameras.astype(np.float64)

    def cm(param):
        theta = param[0] * (PI / 180.0)
        camy = param[3] * np.sin(param[1] * PI / 180.0)
        lens = param[3] * np.cos(param[1] * PI / 180.0)
        camx = lens * np.cos(theta)
        camz = lens * np.sin(theta)
        Zv = np.array([camx, camy, camz])
        Yv = np.array([camy * np.cos(theta + PI), lens, camy * np.sin(theta + PI)])
        Xv = np.cross(Yv, Zv)
        c = np.stack(
            [Xv / np.linalg.norm(Xv), Yv / np.linalg.norm(Yv), Zv / np.linalg.norm(Zv)]
        )
        return c, Zv

    c0, o0 = cm(cams[0])
    M0 = np.linalg.inv(c0.T)
    A, B = [], []
    for v in range(3):
        cv, ov = cm(cams[v])
        A.append(M0 @ cv.T)            # [3,3]
        B.append((o0 - ov) @ cv.T)     # [3]
    return A, B


def _build_affine_plane(cameras: np.ndarray) -> np.ndarray:
    """[P, 40] fp32: per view v, 12 consts at col v*13:
    [a00,a10,a20,b0, -a01,-a11,-a21,-b1, -a02,-a12,-a22,-b2]."""
    A, B = _camera_affines(cameras)
    row = np.zeros(40, np.float32)
    for v in range(3):
        a, b = A[v], B[v]
        base = v * 13
        row[base + 0 : base + 3] = a[:, 0]
        row[base + 3] = b[0]
        row[base + 4 : base + 7] = -a[:, 1]
        row[base + 7] = -b[1]
        row[base + 8 : base + 11] = -a[:, 2]
        row[base + 11] = -b[2]
    return np.tile(row[None, :], (P, 1))


def _build_megatables(img_feat0, img_feat1, img_feat2):
    """Per view: [25088, 512] f16 entries keyed (i0, jp).

    Entry layout (channel-major per level, slot s innermost):
      [  0:128] L0: c=0..15, s=r*4+cw; rows {i0, i0+1}c, wincols {2jp+cw}c
      [128:256] L1: c=0..31, s=r*2+cc; cell (i0//2, jp)
      [256:512] L2: c=0..63, s=r*2+cc; cell (i0//4, jp//2)
    """
    f0 = np.asarray(img_feat0)
    f1 = np.asarray(img_feat1)
    f2 = np.asarray(img_feat2)
    tabs = {}
    for v in range(3):
        F0, F1, F2 = f0[v], f1[v], f2[v]  # [224,224,16] [112,112,32] [56,56,64]
        # L0: [224 rows, 112 jp, 16c, 8s]
        rows = np.stack([np.arange(224), np.minimum(np.arange(224) + 1, 223)], 1)
        cols = np.minimum(
            2 * np.arange(112)[:, None] + np.arange(4)[None, :], 223
        )  # [112, 4]
        t = F0[rows]            # [224, 2, 224, 16]
        t = t[:, :, cols]       # [224, 2, 112, 4, 16]
        l0 = np.transpose(t, (0, 2, 4, 1, 3)).reshape(224, 112, 128)
        # L1 base: [112, 112, 32c, 4s]
        rows1 = np.stack([np.arange(112), np.minimum(np.arange(112) + 1, 111)], 1)
        cols1 = np.stack([np.arange(112), np.minimum(np.arange(112) + 1, 111)], 1)
        t = F1[rows1]           # [112, 2, 112, 32]
        t = t[:, :, cols1]      # [112, 2, 112, 2, 32]
        l1 = np.transpose(t, (0, 2, 4, 1, 3)).reshape(112, 112, 128)
        l1 = np.repeat(l1, 2, axis=0)  # [224, 112, 128]
        # L2 base: [56, 56, 64c, 4s]
        rows2 = np.stack([np.arange(56), np.minimum(np.arange(56) + 1, 55)], 1)
        cols2 = np.stack([np.arange(56), np.minimum(np.arange(56) + 1, 55)], 1)
        t = F2[rows2]           # [56, 2, 56, 64]
        t = t[:, :, cols2]      # [56, 2, 56, 2, 64]
        l2 = np.transpose(t, (0, 2, 4, 1, 3)).reshape(56, 56, 256)
        l2 = np.repeat(np.repeat(l2, 4, axis=0), 2, axis=1)  # [224, 112, 256]
        tab = np.concatenate([l0, l1, l2], axis=2)  # [224, 112, 512]
        tabs[v] = np.ascontiguousarray(tab.reshape(TAB_ROWS, ENT).astype(np.float16))
    return tabs


# ------------------------------------------------------------- device kernel
def emit_body(nc, tc, pools, dram, m_total=M, mt=MT):
    """Emit the whole per-core program inside an open TileContext."""
    nt = m_total // mt
    mt2 = 2 * mt
    nt2 = m_total // mt2
    sc, wpool, gpool, fpool, opool, ipool = (
        pools["sc"], pools["w"], pools["g"], pools["f"], pools["o"], pools["i"],
    )
    coords_d, afp_d, tabs_d, out_d = (
        dram["coords"], dram["afp"], dram["tabs"], dram["out"],
    )

    V = nc.vector
    G = nc.gpsimd
    S = nc.scalar
    IO = nc.sync

    G.load_library(library_config.mlp)
    nidx_reg = G.alloc_register("nidx")
    G.reg_mov(nidx_reg, P * mt)

    # ---- preload
    coords_sb = sc.tile([P, 3, m_total], F32, tag="coords", name="coords_sb")
    IO.dma_start(out=coords_sb[:, :, :], in_=coords_d[:, :, :].transpose([1, 0, 2]))
    afp = sc.tile([P, 40], F32, tag="afp", name="afp_sb")
    IO.dma_start(out=afp[:, :], in_=afp_d[:, :])

    def ap_s(col):  # [P,1] scalar AP
        return afp[:, col : col + 1]

    cx = coords_sb[:, 0, :]
    cy = coords_sb[:, 1, :]
    cz = coords_sb[:, 2, :]

    # ---- whole-core per-point math (fp32, [P, m_total])
    w_tiles = {}   # v -> [P, m_total, 16] fp16 slot weights
    idx_f = sc.tile([P, 3, m_total], F32, tag="idxf", name="idxf")

    def newt(tag, dt=F32, d3=None, pool=sc):
        shape = [P, m_total] if d3 is None else [P, m_total, d3]
        return pool.tile(shape, dt, tag=tag, name=tag)

    for v in range(3):
        base = v * 13
        X = newt("Xs")
        nY = newt("nYs")
        nZ = newt("nZs")
        for out_t, off in ((X, 0), (nY, 4), (nZ, 8)):
            V.tensor_scalar(out_t[:, :], cx, ap_s(base + off + 0), None, ALU.mult)
            V.scalar_tensor_tensor(
                out_t[:, :], cy, ap_s(base + off + 1), out_t[:, :], ALU.mult, ALU.add
            )
            V.scalar_tensor_tensor(
                out_t[:, :], cz, ap_s(base + off + 2), out_t[:, :], ALU.mult, ALU.add
            )
            V.tensor_scalar(out_t[:, :], out_t[:, :], ap_s(base + off + 3), None, ALU.add)
        rz = newt("rzs")
        V.reciprocal(rz[:, :], nZ[:, :])
        h = newt("hs")
        w_ = newt("ws")
        V.tensor_tensor(h[:, :], nY[:, :], rz[:, :], ALU.mult)
        V.tensor_scalar(h[:, :], h[:, :], 248.0, 112.0, ALU.mult, ALU.add)
        V.tensor_scalar(h[:, :], h[:, :], 0.0, 223.0, ALU.max, ALU.min)
        V.tensor_tensor(w_[:, :], X[:, :], rz[:, :], ALU.mult)
        V.tensor_scalar(w_[:, :], w_[:, :], 248.0, 112.0, ALU.mult, ALU.add)
        V.tensor_scalar(w_[:, :], w_[:, :], 0.0, 223.0, ALU.max, ALU.min)

        # floors: i0 = floor(h), fx = h - i0 ; j0 = floor(w), fy = w - j0
        fx = newt("fxs")
        fy = newt("fys")
        i0 = newt("i0s")
        j0 = newt("j0s")
        xi = newt("xis", dt=I32)
        for (xx, x1x, fxx) in ((h, i0, fx), (w_, j0, fy)):
            V.tensor_copy(xi[:, :], xx[:, :])
            V.tensor_copy(x1x[:, :], xi[:, :])
            V.tensor_tensor(fxx[:, :], x1x[:, :], xx[:, :], ALU.is_gt)
            V.tensor_tensor(x1x[:, :], x1x[:, :], fxx[:, :], ALU.subtract)
            V.tensor_tensor(fxx[:, :], xx[:, :], x1x[:, :], ALU.subtract)

        # parities of i0 (mod 2, mod 4) and j0 (mod 2, mod 4)
        pi2 = newt("pi2s")
        pi4 = newt("pi4s")
        pj2 = newt("pj2s")
        pj4 = newt("pj4s")
        for (src, dst, msk) in ((i0, pi2, 1), (i0, pi4, 3), (j0, pj2, 1), (j0, pj4, 3)):
            V.tensor_copy(xi[:, :], src[:, :])
            V.tensor_scalar(xi[:, :], xi[:, :], msk, None, ALU.bitwise_and)
            V.tensor_copy(dst[:, :], xi[:, :])

        # jp = (j0 - pj2)/2 ; idx = i0*112 + jp
        jp = newt("jps")
        V.tensor_tensor(jp[:, :], j0[:, :], pj2[:, :], ALU.subtract)
        V.tensor_scalar(jp[:, :], jp[:, :], 0.5, None, ALU.mult)
        V.scalar_tensor_tensor(
            idx_f[:, v, :], i0[:, :], 112.0, jp[:, :], ALU.mult, ALU.add
        )

        # row weights L0: wx1 = (fx>0) - fx ; wx2 = fx
        wx1 = newt("wx1s")
        wy1 = newt("wy1s")
        V.tensor_scalar(wx1[:, :], fx[:, :], 0.0, None, ALU.is_gt)
        V.tensor_tensor(wx1[:, :], wx1[:, :], fx[:, :], ALU.subtract)
        V.tensor_scalar(wy1[:, :], fy[:, :], 0.0, None, ALU.is_gt)
        V.tensor_tensor(wy1[:, :], wy1[:, :], fy[:, :], ALU.subtract)

        # L0 window-column weights: a0 = wy1*(1-pj2); a1 = wy1*pj2 + fy*(1-pj2);
        # a2 = fy*pj2 ; col 3 = 0
        t1 = newt("i0s")
        t2 = newt("j0s")
        a0 = newt("jps")
        a1 = newt("a1s")
        V.tensor_tensor(t1[:, :], wy1[:, :], pj2[:, :], ALU.mult)
        V.tensor_tensor(a0[:, :], wy1[:, :], t1[:, :], ALU.subtract)
        V.tensor_tensor(t2[:, :], fy[:, :], pj2[:, :], ALU.mult)
        V.tensor_tensor(a1[:, :], t1[:, :], fy[:, :], ALU.add)
        V.tensor_tensor(a1[:, :], a1[:, :], t2[:, :], ALU.subtract)

        # L1 fracs: fx1 = (fx+pi2)/2, fy1 = (fy+pj2)/2; L2: /4 with mod-4 parities
        fx1 = newt("Xs")
        fy1 = newt("nYs")
        fx2 = newt("nZs")
        fy2 = newt("rzs")
        V.tensor_tensor(fx1[:, :], fx[:, :], pi2[:, :], ALU.add)
        V.tensor_scalar(fx1[:, :], fx1[:, :], 0.5, None, ALU.mult)
        V.tensor_tensor(fy1[:, :], fy[:, :], pj2[:, :], ALU.add)
        V.tensor_scalar(fy1[:, :], fy1[:, :], 0.5, None, ALU.mult)
        V.tensor_tensor(fx2[:, :], fx[:, :], pi4[:, :], ALU.add)
        V.tensor_scalar(fx2[:, :], fx2[:, :], 0.25, None, ALU.mult)
        V.tensor_tensor(fy2[:, :], fy[:, :], pj4[:, :], ALU.add)
        V.tensor_scalar(fy2[:, :], fy2[:, :], 0.25, None, ALU.mult)
        wx1_1 = newt("hs")
        wy1_1 = newt("ws")
        wx1_2 = newt("wx12s")
        wy1_2 = newt("wy12s")
        for (f, o) in ((fx1, wx1_1), (fy1, wy1_1), (fx2, wx1_2), (fy2, wy1_2)):
            V.tensor_scalar(o[:, :], f[:, :], 0.0, None, ALU.is_gt)
            V.tensor_tensor(o[:, :], o[:, :], f[:, :], ALU.subtract)

        # weight slots, tiled layout [P, nt, 3, mt, 16] so the combine can
        # merge (view, m) into one uniform-stride dim per tile
        if "w_all" not in w_tiles:
            w_tiles["w_all"] = wpool.tile(
                [P, nt2, 3, mt2, 16], F16, tag="w_all", name="w_all"
            )
        w_all = w_tiles["w_all"]

        def wslot(s):
            return w_all[:, :, v, :, s]

        def asT(x):  # [P, m_total] -> [P, nt2, mt2]
            return x[:, :].rearrange("p (t m) -> p t m", m=mt2)

        # L0 slots 0..7: s = r*4+cw; rows (wx1, fx) x cols (a0, a1, t2, 0)
        for r, rw in ((0, wx1), (1, fx)):
            for c, cw in ((0, a0), (1, a1), (2, t2)):
                V.tensor_tensor(wslot(r * 4 + c), asT(rw), asT(cw), ALU.mult)
        V.tensor_scalar(wslot(3), asT(pj2), 0.0, None, ALU.mult)
        V.tensor_scalar(wslot(7), asT(pj2), 0.0, None, ALU.mult)
        # L1 slots 8..11: (wx1_1*wy1_1, wx1_1*fy1, fx1*wy1_1, fx1*fy1)
        V.tensor_tensor(wslot(8), asT(wx1_1), asT(wy1_1), ALU.mult)
        V.tensor_tensor(wslot(9), asT(wx1_1), asT(fy1), ALU.mult)
        V.tensor_tensor(wslot(10), asT(fx1), asT(wy1_1), ALU.mult)
        V.tensor_tensor(wslot(11), asT(fx1), asT(fy1), ALU.mult)
        # L2 slots 12..15
        V.tensor_tensor(wslot(12), asT(wx1_2), asT(wy1_2), ALU.mult)
        V.tensor_tensor(wslot(13), asT(wx1_2), asT(fy2), ALU.mult)
        V.tensor_tensor(wslot(14), asT(fx2), asT(wy1_2), ALU.mult)
        V.tensor_tensor(wslot(15), asT(fx2), asT(fy2), ALU.mult)

    # cast indices fp32 -> int16
    idx_i = ipool.tile([P, 3, m_total], I16, tag="idxi", name="idxi")
    V.tensor_copy(idx_i[:, :, :].rearrange("p j m -> p (j m)"),
                  idx_f[:, :, :].rearrange("p j m -> p (j m)"))

    # ---- whole-core idx wrap: g=(m*128+p) at [g%16, g//16] per tile chunk,
    # replicated across the 128 partitions
    i32a = gpool.tile([32, 3, m_total, 4], I16, tag="g", name="i32a", bufs=2)
    for r4 in range(4):
        V.tensor_copy(i32a[:, :, :, r4], idx_i[32 * r4 : 32 * (r4 + 1), :, :])
    i16b = gpool.tile([16, 3, m_total, 4], I16, tag="g", name="i16b", bufs=2)
    IO.dma_start(out=i16b[:, :, :, :], in_=i32a[16:32, :, :, :])
    wr_all = ipool.tile([P, 3, 8 * m_total], I16, tag="wr", name="wr")
    wr6 = wr_all.rearrange("p j (t m k r2) -> p j t m k r2", m=mt, k=4, r2=2)
    V.tensor_copy(
        wr6[0:16, :, :, :, :, 0],
        i32a[0:16, :, :, :].rearrange("q j (t m) k -> q j t m k", m=mt),
    )
    V.tensor_copy(
        wr6[0:16, :, :, :, :, 1],
        i16b[:, :, :, :].rearrange("q j (t m) k -> q j t m k", m=mt),
    )
    IO.dma_start(out=wr_all[16:32, :, :], in_=wr_all[0:16, :, :])
    IO.dma_start(out=wr_all[32:64, :, :], in_=wr_all[0:32, :, :])
    IO.dma_start(out=wr_all[64:128, :, :], in_=wr_all[0:64, :, :])

    w_all = w_tiles["w_all"]
    VM = 3 * mt
    SQ3 = 1.0 / float(np.sqrt(3.0))

    # ---- per super-tile (2048 pts): 3x 2048-idx gathers, wide combine,
    # half-tile stats, store
    VM = 3 * mt2
    SQ3 = 1.0 / float(np.sqrt(3.0))
    gq = [0]  # queue = counter%4 keeps Tile DMASW sem lanes queue-pure
    for t in range(nt2):
        gt = gpool.tile([P, 3, mt2, ENT], F16, tag="g", name="g", bufs=2)
        for v in range(3):
            for hh in range(2):
                G.dma_gather(
                    gt[:, v, hh * mt : (hh + 1) * mt, :],
                    tabs_d[v][:, :],
                    wr_all[:, v, (2 * t + hh) * 8 * mt : (2 * t + hh + 1) * 8 * mt],
                    P * mt,
                    nidx_reg,
                    ENT,
                    queue_num=gq[0] % N_QUEUES,
                )
                gq[0] += 1
        gvm = gt[:, :, :, :].rearrange("p v m e -> p (v m) e")
        wt_t = w_all[:, t, :, :, :]
        g0 = gvm[:, :, 0:128].rearrange("p n (c s) -> p n c s", s=8)
        g12 = gvm[:, :, 128:512].rearrange("p n (c s) -> p n c s", s=4)
        g1 = gvm[:, :, 128:256].rearrange("p n (c s) -> p n c s", s=4)
        g2 = gvm[:, :, 256:512].rearrange("p n (c s) -> p n c s", s=4)
        # per-view multiplies so each view combines as soon as its gathers land
        for v in range(3):
            gv = gt[:, v, :, :]
            wv = wt_t[:, v, :, :]
            V.tensor_tensor(
                gv[:, :, 0:128].rearrange("p m (c s) -> p m c s", s=8),
                gv[:, :, 0:128].rearrange("p m (c s) -> p m c s", s=8),
                wv[:, :, 0:8].unsqueeze(2).broadcast_to([P, mt2, 16, 8]),
                ALU.mult)
            V.tensor_tensor(
                gv[:, :, 128:256].rearrange("p m (c s) -> p m c s", s=4),
                gv[:, :, 128:256].rearrange("p m (c s) -> p m c s", s=4),
                wv[:, :, 8:12].unsqueeze(2).broadcast_to([P, mt2, 32, 4]),
                ALU.mult)
            V.tensor_tensor(
                gv[:, :, 256:512].rearrange("p m (c s) -> p m c s", s=4),
                gv[:, :, 256:512].rearrange("p m (c s) -> p m c s", s=4),
                wv[:, :, 12:16].unsqueeze(2).broadcast_to([P, mt2, 64, 4]),
                ALU.mult)
        F_t = fpool.tile([P, 3, mt2, 112], F16, tag="F", name="F")
        Fvm = F_t[:, :, :, :].rearrange("p v m c -> p (v m) c")
        V.tensor_tensor(g0[:, :, :, 0:4], g0[:, :, :, 0:4], g0[:, :, :, 4:8],
                        ALU.add)
        V.tensor_tensor(g0[:, :, :, 0:2], g0[:, :, :, 0:2], g0[:, :, :, 2:4],
                        ALU.add)
        V.tensor_tensor(Fvm[:, :, 0:16], g0[:, :, :, 0], g0[:, :, :, 1], ALU.add)
        V.tensor_tensor(g12[:, :, :, 0:2], g12[:, :, :, 0:2], g12[:, :, :, 2:4],
                        ALU.add)
        V.tensor_tensor(Fvm[:, :, 16:112], g12[:, :, :, 0], g12[:, :, :, 1],
                        ALU.add)

        # ---- stats across views, per half-tile (mt points)
        for hh in range(2):
            hs = slice(hh * mt, (hh + 1) * mt)
            sl = slice(t * mt2 + hh * mt, t * mt2 + (hh + 1) * mt)
            out_t = opool.tile([P, mt, 336], F16, tag="out", name="out_t")
            F0 = F_t[:, 0, hs, :]
            F1 = F_t[:, 1, hs, :]
            F2 = F_t[:, 2, hs, :]
            fmax = out_t[:, :, 0:112]
            V.tensor_tensor(fmax, F0, F1, ALU.max)
            V.tensor_tensor(fmax, fmax, F2, ALU.max)
            ssum = fpool.tile([P, mt, 112], F16, tag="ssum", name="ssum")
            V.tensor_tensor(ssum[:, :, :], F0, F1, ALU.add)
            V.tensor_tensor(ssum[:, :, :], ssum[:, :, :], F2, ALU.add)
            fmean = out_t[:, :, 112:224]
            S.activation(fmean, ssum[:, :, :], ACTF.Copy, scale=1.0 / 3.0)
            sqa = fpool.tile([P, mt, 112], F16, tag="sqa", name="sqa")
            ssq = fpool.tile([P, mt, 112], F16, tag="ssq", name="ssq")
            S.activation(ssq[:, :, :], F0, ACTF.Square, scale=SQ3)
            S.activation(sqa[:, :, :], F1, ACTF.Square, scale=SQ3)
            V.tensor_tensor(ssq[:, :, :], ssq[:, :, :], sqa[:, :, :], ALU.add)
            S.activation(sqa[:, :, :], F2, ACTF.Square, scale=SQ3)
            V.tensor_tensor(ssq[:, :, :], ssq[:, :, :], sqa[:, :, :], ALU.add)
            m2 = fpool.tile([P, mt, 112], F16, tag="sqa", name="m2")
            S.activation(m2[:, :, :], fmean, ACTF.Square)
            V.tensor_tensor(ssq[:, :, :], ssq[:, :, :], m2[:, :, :], ALU.subtract)
            V.tensor_scalar(ssq[:, :, :], ssq[:, :, :], 0.0, None, ALU.max)
            S.activation(out_t[:, :, 224:336], ssq[:, :, :], ACTF.Sqrt)

            IO.dma_start(out=out_d[:, sl, :], in_=out_t[:, :, :])


def build_kernel(m_total=M, mt=MT, tab_rows=TAB_ROWS):
    """Build the Bass module. Returns nc."""
    nc = bacc.Bacc("TRN2", num_swdge_queues=N_QUEUES, dynamic_dma_scratch_size=16384)
    coords = nc.dram_tensor("coords", [3, P, m_total], F32, kind="ExternalInput")
    afp = nc.dram_tensor("afp", [P, 40], F32, kind="ExternalInput")
    tabs = {}
    for v in range(3):
        tabs[v] = nc.dram_tensor(
            f"tab{v}", [tab_rows, ENT], F16, kind="ExternalInput"
        )
    out = nc.dram_tensor("out", [P, m_total, 336], F16, kind="ExternalOutput")

    with nc.allow_low_precision("fp16 sampling kernel"), TileContext(nc) as tc:
        import contextlib

        stack = contextlib.ExitStack()
        pools = {
            "sc": stack.enter_context(tc.tile_pool(name="sc", bufs=1)),
            "w": stack.enter_context(tc.tile_pool(name="w", bufs=1)),
            "g": stack.enter_context(tc.tile_pool(name="g", bufs=2)),
            "f": stack.enter_context(tc.tile_pool(name="f", bufs=2)),
            "o": stack.enter_context(tc.tile_pool(name="o", bufs=2)),
            "i": stack.enter_context(tc.tile_pool(name="i", bufs=1)),
        }
        dram = {
            "coords": coords.ap(),
            "afp": afp.ap(),
            "tabs": {v: t.ap() for v, t in tabs.items()},
            "out": out.ap(),
        }
        with stack:
            emit_body(nc, tc, pools, dram, m_total=m_total, mt=mt)
    nc.compile()
    return nc


# ------------------------------------------------------------------ frontend
_NC_CACHE = {}
TRACE = False
LAST_RES = [None]


def _get_nc():
    if "nc" not in _NC_CACHE:
        _NC_CACHE["nc"] = build_kernel()
    return _NC_CACHE["nc"]


def kernel(coord, img_feat0, img_feat1, img_feat2, cameras):
    coord = np.asarray(coord, np.float32)
    afp = _build_affine_plane(np.asarray(cameras, np.float32))
    tabs = _build_megatables(img_feat0, img_feat1, img_feat2)

    nc = _get_nc()
    in_maps = []
    for k in range(N_CORES):
        shard = coord[k * N_CORE_PTS : (k + 1) * N_CORE_PTS]  # [32768, 3]
        cs = np.ascontiguousarray(
            shard.reshape(P, M, 3).transpose(2, 0, 1)
        )  # [3, P, M]
        im = {"coords": cs, "afp": afp}
        for v in range(3):
            im[f"tab{v}"] = tabs[v]
        in_maps.append(im)

    res = run_bass_kernel_spmd(
        nc, in_maps, core_ids=list(range(N_CORES)), trace=TRACE
    )
    LAST_RES[0] = res
    stats = np.concatenate(
        [res.results[k]["out"].reshape(N_CORE_PTS, 336) for k in range(N_CORES)], 0
    ).astype(np.float32)
    return np.concatenate([coord, stats], axis=1)
